# revision 22
# baseline (speedup 1.0000x reference)
"""Trainium2 Bass kernel for batched bilinear (general) attention.

Reference computation (all fp32):
    psi = einsum("bth,ah->bta", h_enc, W_psi) + b_psi        # [B, T, A]
    phi = einsum("qbh,ah->qba", h_dec, W_phi) + b_phi        # [Q, B, A]
    e   = einsum("bta,qba->btq", psi, phi)                   # [B, T, Q]
    a   = softmax(e, axis=1)                                 # over T
    c   = einsum("bth,btq->bqh", h_enc, a)                   # [B, Q, H]

Algebraic refactor: e[b,t,q] = enc_t . M . dec_q + enc_t . u + (per-q const)
with M = W_psi^T @ W_phi [H,H], u = W_psi^T @ b_phi.  Per-q-column constants
are softmax-invariant and dropped.  The host folds weights into
Z[b] = M @ dec_b^T + u [H, Q] (tiny); the device computes e = enc @ Z,
softmax over T, and c = p^T @ enc.

Precision: e is computed as zh.enc16 + (zl.enc16)/SC with zh = f16(Z),
zl = f16((Z - zh)*SC) packed side by side in one [128-col] stationary
operand, so both channels come out of a single matmul pass (psum rows 0:64 =
main, 64:128 = correction).  Measured end-to-end rel-err 6e-3 vs the 2e-2
gate.

DMA-diet: enc is loaded from HBM ONCE per batch (f16, H-major "encT" form
for the e-phase).  The T-major copy needed by the c-phase is produced
on-chip by PE transposes hidden under the DMA stream for most tiles; only
`nloads[b]` of the 16 t-tiles per batch are loaded pre-transposed from HBM
(to balance PE vs DMA).  The last batch uses nloads=0 so its c-phase never
waits on DMA at the kernel tail.

Sharding: data-parallel over batch B=16 across 8 cores (2 per core).
"""

import functools
import os
import sys

import numpy as np

for _p in ("/opt/trn_rl_repo", "/root/.axon_site/_ro/trn_rl_repo"):
    if os.path.isdir(_p) and _p not in sys.path:
        sys.path.append(_p)

B, T, Q, H = 16, 2048, 64, 1024
NCORES = 8
BL = B // NCORES  # batches per core
KT = H // 128  # 8 contraction tiles for e
NT = T // 128  # 16 t-tiles
NC_CHUNK = T // 512  # 4 psum chunks along T for e
GE = int(os.environ.get("ATTN_GE", "1"))  # k-tiles per encT DMA transfer
GC = 2  # t-tiles per encN DMA (1 MB transfers, 4 KB contiguous/partition)
SC = 2048.0  # 2^11 scale for the zl correction channel

NLOADS = tuple(
    int(x) for x in os.environ.get("ATTN_NLOADS", "0,16").split(",")
)
DMA_SPREAD = int(os.environ.get("ATTN_DMA_SPREAD", "2"))
# SAFE=1: f32-PSUM transposes via regular matmul-by-identity (baseline-proven
# pattern); SAFE=0: f16-PSUM is_transpose path (faster, less PSUM)
SAFE = int(os.environ.get("ATTN_SAFE", "0"))


@functools.lru_cache(maxsize=4)
def _build(loop_n: int = 1, nloads: tuple = NLOADS, dma_spread: int = DMA_SPREAD, safe: int = SAFE):
    import contextlib

    import concourse.mybir as mybir
    import concourse.tile as tile
    from concourse import bacc
    from concourse.bass import ts
    from concourse.masks import make_identity

    f32 = mybir.dt.float32
    f16 = mybir.dt.float16

    nc = bacc.Bacc(
        "TRN2",
        target_bir_lowering=False,
        debug=False,
        enable_asserts=False,
        num_devices=NCORES,
    )

    encT_d = nc.dram_tensor(
        "encT", [BL, KT // GE, 128, GE * T], f16, kind="ExternalInput"
    )
    n_enc_dma = sum(nloads) // GC
    if n_enc_dma:
        encN_d = nc.dram_tensor(
            "encN", [n_enc_dma, 128, GC * H], f16, kind="ExternalInput"
        )
    z_d = nc.dram_tensor("z", [BL, 128, KT, 2 * Q], f16, kind="ExternalInput")
    c_d = nc.dram_tensor("c", [BL, Q, H], f32, kind="ExternalOutput")

    with tile.TileContext(nc) as tc:
        rings = [nc.sync, nc.scalar, nc.gpsimd, nc.vector][: max(2, dma_spread)]

        def dma(ring, out, in_):
            rings[ring % len(rings)].dma_start(out=out, in_=in_)

        with (
            tc.tile_pool(name="encT", bufs=10) as p_encT,
            tc.tile_pool(name="encN", bufs=2) as p_encN,
            tc.tile_pool(name="z", bufs=2) as p_z,
            tc.tile_pool(name="eT", bufs=2) as p_eT,
            tc.tile_pool(name="pT", bufs=2) as p_pT,
            tc.tile_pool(name="pN", bufs=2) as p_pN,
            tc.tile_pool(name="outs", bufs=2) as p_out,
            tc.tile_pool(name="stats", bufs=12) as p_stats,
            tc.tile_pool(name="singles", bufs=1) as p_singles,
            tc.tile_pool(name="ps", bufs=8, space="PSUM") as ps,
        ):
            ident128 = p_singles.tile([128, 128], f16)
            make_identity(nc, ident128)
            ident64 = p_singles.tile([64, 64], f32 if safe else f16)
            make_identity(nc, ident64)

            loop_ctx = (
                tc.For_i(0, loop_n, 1) if loop_n > 1 else contextlib.nullcontext()
            )
            with loop_ctx:
                z_ts, encN_sbs = [], []
                for b in range(BL):
                    z_t = p_z.tile([128, KT, 2 * Q], f16, tag="z")
                    dma(1, z_t[:], z_d.ap()[b])
                    z_ts.append(z_t)
                    encN_sb = p_encN.tile([128, NT, H], f16, tag="encN", name=f"encN_{b}")
                    encN_sbs.append(encN_sb)

                enc_dma_i = [0]

                def load_encN(b):
                    encN_sb = encN_sbs[b]
                    for i in range(nloads[b] // GC):
                        dma(
                            0,
                            encN_sb[:, GC * i : GC * (i + 1), :],
                            encN_d.ap()[enc_dma_i[0]],
                        )
                        enc_dma_i[0] += 1

                def phase_E(b):
                    """e^T[b] = Zpk[b]^T @ encT[b]; transpose ntr t-tiles of enc."""
                    z_t = z_ts[b]
                    encN_sb = encN_sbs[b]
                    nload = nloads[b]
                    ntr = NT - nload
                    e_pss = [
                        ps.tile([128, 512], f32, tag="ps", name=f"e_ps_{b}_{ci}")
                        for ci in range(NC_CHUNK)
                    ]
                    for kk in range(KT // GE):
                        encT_g = p_encT.tile([128, GE * T], f16, tag="encT")
                        dma(0, encT_g[:], encT_d.ap()[b, kk])
                        for g in range(GE):
                            k = kk * GE + g
                            for ci in range(NC_CHUNK):
                                nc.tensor.matmul(
                                    e_pss[ci][:],
                                    lhsT=z_t[:, k, :],
                                    rhs=encT_g[:, ts(g * NC_CHUNK + ci, 512)],
                                    start=(k == 0),
                                    stop=(k == KT - 1),
                                    skip_group_check=True,
                                )
                            # on-chip production of the T-major enc copy
                            gsz = 4 if safe else 8
                            for gi, h0 in enumerate(range(0, ntr, gsz)):
                                hn = min(gsz, ntr - h0)
                                tp = ps.tile(
                                    [128, hn, 128], f32 if safe else f16,
                                    tag="ps", name=f"tp_{b}_{k}_{h0}",
                                )
                                for j in range(hn):
                                    tt = nload + h0 + j
                                    src_ap = encT_g[
                                        :, g * T + tt * 128 : g * T + (tt + 1) * 128
                                    ]
                                    if safe:
                                        # out = encT_slice^T @ I = transposed
                                        # tile, in ordinary f32 PSUM
                                        nc.tensor.matmul(
                                            tp[:, j, :],
                                            lhsT=src_ap,
                                            rhs=ident128[:],
                                            start=True,
                                            stop=True,
                                            skip_group_check=True,
                                        )
                                    else:
                                        nc.tensor.matmul(
                                            tp[:, j, :],
                                            lhsT=src_ap,
                                            rhs=ident128[:],
                                            is_transpose=True,
                                            start=True,
                                            stop=True,
                                            skip_group_check=True,
                                        )
                                dst = encN_sb[
                                    :, nload + h0 : nload + h0 + hn, ts(k, 128)
                                ]
                                if (k + gi) % 2 == 0:
                                    nc.vector.tensor_copy(out=dst, in_=tp[:])
                                else:
                                    nc.scalar.copy(out=dst, in_=tp[:])
                    return e_pss

                def phase_S(b, e_pss):
                    """softmax stats over T; p in f16, correction folded in."""
                    eT = p_eT.tile([64, T], f32, tag="eT")
                    for ci in range(NC_CHUNK):
                        nc.scalar.activation(
                            out=eT[:, ts(ci, 512)],
                            in_=e_pss[ci][64:128, :],
                            func=mybir.ActivationFunctionType.Copy,
                            bias=0.0,
                            scale=1.0 / SC,
                        )
                        nc.vector.tensor_add(
                            eT[:, ts(ci, 512)], eT[:, ts(ci, 512)], e_pss[ci][0:64, :]
                        )
                    negm = p_stats.tile([64, 1], f32, tag="negm")
                    nc.vector.reduce_max(
                        out=negm[:], in_=eT[:], axis=mybir.AxisListType.X, negate=True
                    )
                    pT = p_pT.tile([64, T], f16 if not safe else f32, tag="pT")
                    s_sum = p_stats.tile([64, 1], f32, tag="s")
                    nc.scalar.activation(
                        out=pT[:],
                        in_=eT[:],
                        func=mybir.ActivationFunctionType.Exp,
                        bias=negm[:],
                        scale=1.0,
                        accum_out=s_sum[:],
                    )
                    r = p_stats.tile([64, 1], f32, tag="r")
                    nc.vector.reciprocal(out=r[:], in_=s_sum[:])
                    return pT, r

                def phase_C(b, pT, r):
                    """c[b] = (p^T @ encN) * r, transposed t-tiles first."""
                    nload = nloads[b]
                    encN_sb = encN_sbs[b]
                    pN = p_pN.tile([128, NT, Q], f16, tag="pN")
                    for tg in range(NT // 4):
                        trp = ps.tile(
                            [128, 4, Q], f32 if safe else f16,
                            tag="ps", name=f"trp_{b}_{tg}",
                        )
                        for j in range(4):
                            tt = tg * 4 + j
                            if safe:
                                nc.tensor.transpose(
                                    out=trp[:, j, :],
                                    in_=pT[:, ts(tt, 128)],
                                    identity=ident64[:],
                                )
                            else:
                                nc.tensor.matmul(
                                    trp[:, j, :],
                                    lhsT=pT[:, ts(tt, 128)],
                                    rhs=ident64[:],
                                    is_transpose=True,
                                    start=True,
                                    stop=True,
                                    skip_group_check=True,
                                )
                        nc.vector.tensor_copy(
                            out=pN[:, tg * 4 : (tg + 1) * 4, :], in_=trp[:]
                        )
                    c_ps0 = ps.tile([64, 512], f32, tag="ps", name=f"c0_{b}")
                    c_ps1 = ps.tile([64, 512], f32, tag="ps", name=f"c1_{b}")
                    order = list(range(nload, NT)) + list(range(nload))
                    for i, tt in enumerate(order):
                        nc.tensor.matmul(
                            c_ps0[:],
                            lhsT=pN[:, tt, :],
                            rhs=encN_sb[:, tt, 0:512],
                            start=(i == 0),
                            stop=(i == NT - 1),
                            skip_group_check=True,
                        )
                        nc.tensor.matmul(
                            c_ps1[:],
                            lhsT=pN[:, tt, :],
                            rhs=encN_sb[:, tt, 512:1024],
                            start=(i == 0),
                            stop=(i == NT - 1),
                            skip_group_check=True,
                        )
                    out_t = p_out.tile([64, H], f32, tag="out")
                    nc.vector.tensor_scalar_mul(out_t[:, 0:512], c_ps0[:], r[:])
                    nc.vector.tensor_scalar_mul(out_t[:, 512:1024], c_ps1[:], r[:])
                    # sync ring: all input loads precede the outs in SP's
                    # FIFO, so the sequencer-side wait for out_t blocks nothing
                    dma(0, c_d.ap()[b], out_t[:])

                # PE warm-up: data-independent transposes ramp the tensor
                # engine to its max p-state while the first DMAs land
                n_warm = int(os.environ.get("ATTN_WARM", "16"))
                if n_warm:
                    warm_ps = ps.tile(
                        [128, 128], f32 if safe else f16, tag="ps", name="warm"
                    )
                    for _ in range(n_warm):
                        kw = {} if safe else {"is_transpose": True}
                        nc.tensor.matmul(
                            warm_ps[:],
                            lhsT=ident128[:],
                            rhs=ident128[:],
                            start=True,
                            stop=True,
                            skip_group_check=True,
                            **kw,
                        )

                # PE order: E0, E1, C0, C1 — softmax S(b) runs on ACT/DVE in
                # the shadow of the next phase's PE stream, so PE never stalls
                # on it.  DMA ring 0 order: encT b0, encT b1, encN b1.
                e_pss0 = phase_E(0)
                load_encN(0)
                pT0, r0 = phase_S(0, e_pss0)
                e_pss1 = phase_E(1)
                load_encN(1)
                phase_C(0, pT0, r0)
                pT1, r1 = phase_S(1, e_pss1)
                phase_C(1, pT1, r1)

    nc.compile()
    return nc


def _host_prep(h_enc, h_dec, W_psi, b_psi, W_phi, b_phi, nloads: tuple = NLOADS):
    h_enc = np.asarray(h_enc, dtype=np.float32)
    h_dec = np.asarray(h_dec, dtype=np.float32)
    W_psi = np.asarray(W_psi, dtype=np.float64)
    W_phi = np.asarray(W_phi, dtype=np.float64)
    b_phi = np.asarray(b_phi, dtype=np.float64)

    # M = W_psi^T @ W_phi [H, H];  u = W_psi^T @ b_phi [H]
    M = W_psi.T @ W_phi
    u = W_psi.T @ b_phi
    # Z[b, h, q] = sum_k M[h, k] * h_dec[q, b, k] + u[h]
    dec_r = h_dec.astype(np.float64).transpose(2, 1, 0).reshape(H, B * Q)
    Z = (M @ dec_r).reshape(H, B, Q).transpose(1, 0, 2) + u[None, :, None]
    Z = np.ascontiguousarray(Z, dtype=np.float32)  # [B, H, Q]

    def tile_i(x, g):  # [B, G*g*128, W] -> [B, G, 128, g*W] interleaved
        Bn, R, W = x.shape
        G = R // (g * 128)
        return np.ascontiguousarray(
            x.reshape(Bn, G, g, 128, W).transpose(0, 1, 3, 2, 4).reshape(
                Bn, G, 128, g * W
            )
        )

    encT = np.ascontiguousarray(h_enc.transpose(0, 2, 1))  # [B, H, T] fp32
    arrays = {"encT": tile_i(encT.astype(np.float16), GE)}

    zh = Z.astype(np.float16)
    zl = ((Z - zh.astype(np.float32)) * SC).astype(np.float16)
    zpk = np.concatenate([zh, zl], axis=2)  # [B, H, 2Q]
    arrays["z"] = np.ascontiguousarray(
        zpk.reshape(B, KT, 128, 2 * Q).transpose(0, 2, 1, 3)
    )  # [B, 128, KT, 2Q]

    if sum(nloads):
        encN16 = h_enc.astype(np.float16)  # [B, T, H]
        pieces = []
        for core in range(NCORES):
            for bl, nload in enumerate(nloads):
                if nload:
                    bglob = core * BL + bl
                    pieces.append(
                        tile_i(encN16[bglob : bglob + 1, : nload * 128, :], GC)[0]
                    )
        # [NCORES * sum(nloads)//GC, 128, GC*H]
        arrays["encN"] = np.ascontiguousarray(np.stack(pieces, 0).reshape(
            NCORES, -1, 128, GC * H
        ))
    return arrays


def _in_maps(arrays):
    maps = []
    for i in range(NCORES):
        m = {}
        for k, v in arrays.items():
            if k == "encN":
                m[k] = v[i]
            else:
                m[k] = v[i * BL : (i + 1) * BL]
        maps.append(m)
    return maps


def kernel(h_enc, h_dec, W_psi, b_psi, W_phi, b_phi):
    from concourse.bass_utils import run_bass_kernel_spmd

    arrays = _host_prep(h_enc, h_dec, W_psi, b_psi, W_phi, b_phi)
    nc = _build()
    res = run_bass_kernel_spmd(nc, _in_maps(arrays), core_ids=list(range(NCORES)))
    out = np.concatenate([res.results[i]["c"] for i in range(NCORES)], axis=0)
    return np.ascontiguousarray(out, dtype=np.float32)


# revision 26
# speedup vs baseline: 1.0094x; 1.0094x over previous
"""Trainium2 Bass kernel for batched bilinear (general) attention.

Reference computation (all fp32):
    psi = einsum("bth,ah->bta", h_enc, W_psi) + b_psi        # [B, T, A]
    phi = einsum("qbh,ah->qba", h_dec, W_phi) + b_phi        # [Q, B, A]
    e   = einsum("bta,qba->btq", psi, phi)                   # [B, T, Q]
    a   = softmax(e, axis=1)                                 # over T
    c   = einsum("bth,btq->bqh", h_enc, a)                   # [B, Q, H]

Algebraic refactor: e[b,t,q] = enc_t . M . dec_q + enc_t . u + (per-q const)
with M = W_psi^T @ W_phi [H,H], u = W_psi^T @ b_phi.  Per-q-column constants
are softmax-invariant and dropped.  The host folds weights into
Z[b] = M @ dec_b^T + u [H, Q] (tiny); the device computes e = enc @ Z,
softmax over T, and c = p^T @ enc.

Precision: e is computed as zh.enc16 + (zl.enc16)/SC with zh = f16(Z),
zl = f16((Z - zh)*SC) packed side by side in one [128-col] stationary
operand, so both channels come out of a single matmul pass (psum rows 0:64 =
main, 64:128 = correction).  Measured end-to-end rel-err 6e-3 vs the 2e-2
gate.

DMA-diet: enc is loaded from HBM ONCE per batch (f16, H-major "encT" form
for the e-phase).  The T-major copy needed by the c-phase is produced
on-chip by PE transposes hidden under the DMA stream for most tiles; only
`nloads[b]` of the 16 t-tiles per batch are loaded pre-transposed from HBM
(to balance PE vs DMA).  The last batch uses nloads=0 so its c-phase never
waits on DMA at the kernel tail.

Sharding: data-parallel over batch B=16 across 8 cores (2 per core).
"""

import functools
import os
import sys

import numpy as np

for _p in ("/opt/trn_rl_repo", "/root/.axon_site/_ro/trn_rl_repo"):
    if os.path.isdir(_p) and _p not in sys.path:
        sys.path.append(_p)

B, T, Q, H = 16, 2048, 64, 1024
NCORES = 8
BL = B // NCORES  # batches per core
KT = H // 128  # 8 contraction tiles for e
NT = T // 128  # 16 t-tiles
NC_CHUNK = T // 512  # 4 psum chunks along T for e
GE = int(os.environ.get("ATTN_GE", "1"))  # k-tiles per encT DMA transfer
GC = 2  # t-tiles per encN DMA (1 MB transfers, 4 KB contiguous/partition)
SC = 2048.0  # 2^11 scale for the zl correction channel

NLOADS = tuple(
    int(x) for x in os.environ.get("ATTN_NLOADS", "0,16").split(",")
)
DMA_SPREAD = int(os.environ.get("ATTN_DMA_SPREAD", "2"))
# SAFE=1: f32-PSUM transposes via regular matmul-by-identity (baseline-proven
# pattern); SAFE=0: f16-PSUM is_transpose path (faster, less PSUM)
SAFE = int(os.environ.get("ATTN_SAFE", "0"))


@functools.lru_cache(maxsize=4)
def _build(loop_n: int = 1, nloads: tuple = NLOADS, dma_spread: int = DMA_SPREAD, safe: int = SAFE):
    import contextlib

    import concourse.mybir as mybir
    import concourse.tile as tile
    from concourse import bacc
    from concourse.bass import ts
    from concourse.masks import make_identity

    f32 = mybir.dt.float32
    f16 = mybir.dt.float16

    nc = bacc.Bacc(
        "TRN2",
        target_bir_lowering=False,
        debug=False,
        enable_asserts=False,
        num_devices=NCORES,
    )

    encT_d = nc.dram_tensor(
        "encT", [BL, KT // GE, 128, GE * T], f16, kind="ExternalInput"
    )
    n_enc_dma = sum(nloads) // GC
    if n_enc_dma:
        encN_d = nc.dram_tensor(
            "encN", [n_enc_dma, 128, GC * H], f16, kind="ExternalInput"
        )
    z_d = nc.dram_tensor("z", [BL, 128, KT, 2 * Q], f16, kind="ExternalInput")
    c_d = nc.dram_tensor("c", [BL, Q, H], f32, kind="ExternalOutput")

    with tile.TileContext(nc) as tc:
        rings = [nc.sync, nc.scalar, nc.gpsimd, nc.vector][: max(2, dma_spread)]

        def dma(ring, out, in_):
            rings[ring % len(rings)].dma_start(out=out, in_=in_)

        with (
            tc.tile_pool(name="encT", bufs=int(os.environ.get("ATTN_ETBUFS", "16"))) as p_encT,
            tc.tile_pool(name="encN", bufs=2) as p_encN,
            tc.tile_pool(name="z", bufs=2) as p_z,
            tc.tile_pool(name="eT", bufs=2) as p_eT,
            tc.tile_pool(name="pT", bufs=2) as p_pT,
            tc.tile_pool(name="pN", bufs=2) as p_pN,
            tc.tile_pool(name="outs", bufs=2) as p_out,
            tc.tile_pool(name="stats", bufs=12) as p_stats,
            tc.tile_pool(name="singles", bufs=1) as p_singles,
            tc.tile_pool(name="ps", bufs=8, space="PSUM") as ps,
        ):
            ident128 = p_singles.tile([128, 128], f16)
            make_identity(nc, ident128)
            ident64 = p_singles.tile([64, 64], f32 if safe else f16)
            make_identity(nc, ident64)

            loop_ctx = (
                tc.For_i(0, loop_n, 1) if loop_n > 1 else contextlib.nullcontext()
            )
            with loop_ctx:
                z_ts, encN_sbs = [], []
                for b in range(BL):
                    z_t = p_z.tile([128, KT, 2 * Q], f16, tag="z")
                    dma(1, z_t[:], z_d.ap()[b])
                    z_ts.append(z_t)
                    encN_sb = p_encN.tile([128, NT, H], f16, tag="encN", name=f"encN_{b}")
                    encN_sbs.append(encN_sb)

                enc_dma_i = [0]

                def load_encN(b):
                    encN_sb = encN_sbs[b]
                    for i in range(nloads[b] // GC):
                        dma(
                            0,
                            encN_sb[:, GC * i : GC * (i + 1), :],
                            encN_d.ap()[enc_dma_i[0]],
                        )
                        enc_dma_i[0] += 1

                def phase_E(b):
                    """e^T[b] = Zpk[b]^T @ encT[b]; transpose ntr t-tiles of enc."""
                    z_t = z_ts[b]
                    encN_sb = encN_sbs[b]
                    nload = nloads[b]
                    ntr = NT - nload
                    e_pss = [
                        ps.tile([128, 512], f32, tag="ps", name=f"e_ps_{b}_{ci}")
                        for ci in range(NC_CHUNK)
                    ]
                    for kk in range(KT // GE):
                        encT_g = p_encT.tile([128, GE * T], f16, tag="encT")
                        dma(0, encT_g[:], encT_d.ap()[b, kk])
                        for g in range(GE):
                            k = kk * GE + g
                            for ci in range(NC_CHUNK):
                                nc.tensor.matmul(
                                    e_pss[ci][:],
                                    lhsT=z_t[:, k, :],
                                    rhs=encT_g[:, ts(g * NC_CHUNK + ci, 512)],
                                    start=(k == 0),
                                    stop=(k == KT - 1),
                                    skip_group_check=True,
                                )
                            # on-chip production of the T-major enc copy
                            gsz = 4 if safe else 8
                            for gi, h0 in enumerate(range(0, ntr, gsz)):
                                hn = min(gsz, ntr - h0)
                                tp = ps.tile(
                                    [128, hn, 128], f32 if safe else f16,
                                    tag="ps", name=f"tp_{b}_{k}_{h0}",
                                )
                                for j in range(hn):
                                    tt = nload + h0 + j
                                    src_ap = encT_g[
                                        :, g * T + tt * 128 : g * T + (tt + 1) * 128
                                    ]
                                    if safe:
                                        # out = encT_slice^T @ I = transposed
                                        # tile, in ordinary f32 PSUM
                                        nc.tensor.matmul(
                                            tp[:, j, :],
                                            lhsT=src_ap,
                                            rhs=ident128[:],
                                            start=True,
                                            stop=True,
                                            skip_group_check=True,
                                        )
                                    else:
                                        nc.tensor.matmul(
                                            tp[:, j, :],
                                            lhsT=src_ap,
                                            rhs=ident128[:],
                                            is_transpose=True,
                                            start=True,
                                            stop=True,
                                            skip_group_check=True,
                                        )
                                dst = encN_sb[
                                    :, nload + h0 : nload + h0 + hn, ts(k, 128)
                                ]
                                if (k + gi) % 2 == 0:
                                    nc.vector.tensor_copy(out=dst, in_=tp[:])
                                else:
                                    nc.scalar.copy(out=dst, in_=tp[:])
                    return e_pss

                def phase_S(b, e_pss):
                    """softmax stats over T; p in f16, correction folded in."""
                    eT = p_eT.tile([64, T], f32, tag="eT")
                    for ci in range(NC_CHUNK):
                        nc.scalar.activation(
                            out=eT[:, ts(ci, 512)],
                            in_=e_pss[ci][64:128, :],
                            func=mybir.ActivationFunctionType.Copy,
                            bias=0.0,
                            scale=1.0 / SC,
                        )
                        nc.vector.tensor_add(
                            eT[:, ts(ci, 512)], eT[:, ts(ci, 512)], e_pss[ci][0:64, :]
                        )
                    negm = p_stats.tile([64, 1], f32, tag="negm")
                    nc.vector.reduce_max(
                        out=negm[:], in_=eT[:], axis=mybir.AxisListType.X, negate=True
                    )
                    pT = p_pT.tile([64, T], f16 if not safe else f32, tag="pT")
                    s_sum = p_stats.tile([64, 1], f32, tag="s")
                    nc.scalar.activation(
                        out=pT[:],
                        in_=eT[:],
                        func=mybir.ActivationFunctionType.Exp,
                        bias=negm[:],
                        scale=1.0,
                        accum_out=s_sum[:],
                    )
                    r = p_stats.tile([64, 1], f32, tag="r")
                    nc.vector.reciprocal(out=r[:], in_=s_sum[:])
                    return pT, r

                def phase_C(b, pT, r):
                    """c[b] = (p^T @ encN) * r, transposed t-tiles first."""
                    nload = nloads[b]
                    encN_sb = encN_sbs[b]
                    pN = p_pN.tile([128, NT, Q], f16, tag="pN")
                    for tg in range(NT // 4):
                        trp = ps.tile(
                            [128, 4, Q], f32 if safe else f16,
                            tag="ps", name=f"trp_{b}_{tg}",
                        )
                        for j in range(4):
                            tt = tg * 4 + j
                            if safe:
                                nc.tensor.transpose(
                                    out=trp[:, j, :],
                                    in_=pT[:, ts(tt, 128)],
                                    identity=ident64[:],
                                )
                            else:
                                nc.tensor.matmul(
                                    trp[:, j, :],
                                    lhsT=pT[:, ts(tt, 128)],
                                    rhs=ident64[:],
                                    is_transpose=True,
                                    start=True,
                                    stop=True,
                                    skip_group_check=True,
                                )
                        nc.vector.tensor_copy(
                            out=pN[:, tg * 4 : (tg + 1) * 4, :], in_=trp[:]
                        )
                    # both H-halves accumulate in ONE psum bank: half 0 in
                    # partitions 0:64, half 1 in 64:128 via tile_position
                    c_ps = ps.tile([128, 512], f32, tag="ps", name=f"c_{b}")
                    order = list(range(nload, NT)) + list(range(nload))
                    for i, tt in enumerate(order):
                        nc.tensor.matmul(
                            c_ps[0:64, :],
                            lhsT=pN[:, tt, :],
                            rhs=encN_sb[:, tt, 0:512],
                            start=(i == 0),
                            stop=(i == NT - 1),
                            skip_group_check=True,
                        )
                        nc.tensor.matmul(
                            c_ps[64:128, :],
                            lhsT=pN[:, tt, :],
                            rhs=encN_sb[:, tt, 512:1024],
                            start=(i == 0),
                            stop=(i == NT - 1),
                            tile_position=(0, 64),
                            skip_group_check=True,
                        )
                    out_t = p_out.tile([64, H], f32, tag="out")
                    nc.vector.tensor_scalar_mul(out_t[:, 0:512], c_ps[0:64, :], r[:])
                    nc.vector.tensor_scalar_mul(
                        out_t[:, 512:1024], c_ps[64:128, :], r[:]
                    )
                    # Pool/gpsimd SWDGE ring: a waiting out-store here blocks
                    # nothing — under For_i an out on the sync ring would stall
                    # the NEXT iteration's encT loads behind it in SP's FIFO
                    nc.gpsimd.dma_start(out=c_d.ap()[b], in_=out_t[:])

                # PE warm-up: data-independent transposes ramp the tensor
                # engine to its max p-state while the first DMAs land
                n_warm = int(os.environ.get("ATTN_WARM", "16"))
                if n_warm:
                    warm_ps = ps.tile(
                        [128, 128], f32 if safe else f16, tag="ps", name="warm"
                    )
                    for _ in range(n_warm):
                        kw = {} if safe else {"is_transpose": True}
                        nc.tensor.matmul(
                            warm_ps[:],
                            lhsT=ident128[:],
                            rhs=ident128[:],
                            start=True,
                            stop=True,
                            skip_group_check=True,
                            **kw,
                        )

                # PE order: E0, E1, C0, C1 — softmax S(b) runs on ACT/DVE in
                # the shadow of the next phase's PE stream, so PE never stalls
                # on it.  DMA ring 0 order: encT b0, encT b1, encN b1.
                e_pss0 = phase_E(0)
                load_encN(0)
                pT0, r0 = phase_S(0, e_pss0)
                e_pss1 = phase_E(1)
                load_encN(1)
                phase_C(0, pT0, r0)
                pT1, r1 = phase_S(1, e_pss1)
                phase_C(1, pT1, r1)

    nc.compile()
    return nc


def _host_prep(h_enc, h_dec, W_psi, b_psi, W_phi, b_phi, nloads: tuple = NLOADS):
    h_enc = np.asarray(h_enc, dtype=np.float32)
    h_dec = np.asarray(h_dec, dtype=np.float32)
    W_psi = np.asarray(W_psi, dtype=np.float64)
    W_phi = np.asarray(W_phi, dtype=np.float64)
    b_phi = np.asarray(b_phi, dtype=np.float64)

    # M = W_psi^T @ W_phi [H, H];  u = W_psi^T @ b_phi [H]
    M = W_psi.T @ W_phi
    u = W_psi.T @ b_phi
    # Z[b, h, q] = sum_k M[h, k] * h_dec[q, b, k] + u[h]
    dec_r = h_dec.astype(np.float64).transpose(2, 1, 0).reshape(H, B * Q)
    Z = (M @ dec_r).reshape(H, B, Q).transpose(1, 0, 2) + u[None, :, None]
    Z = np.ascontiguousarray(Z, dtype=np.float32)  # [B, H, Q]

    def tile_i(x, g):  # [B, G*g*128, W] -> [B, G, 128, g*W] interleaved
        Bn, R, W = x.shape
        G = R // (g * 128)
        return np.ascontiguousarray(
            x.reshape(Bn, G, g, 128, W).transpose(0, 1, 3, 2, 4).reshape(
                Bn, G, 128, g * W
            )
        )

    encT = np.ascontiguousarray(h_enc.transpose(0, 2, 1))  # [B, H, T] fp32
    arrays = {"encT": tile_i(encT.astype(np.float16), GE)}

    zh = Z.astype(np.float16)
    zl = ((Z - zh.astype(np.float32)) * SC).astype(np.float16)
    zpk = np.concatenate([zh, zl], axis=2)  # [B, H, 2Q]
    arrays["z"] = np.ascontiguousarray(
        zpk.reshape(B, KT, 128, 2 * Q).transpose(0, 2, 1, 3)
    )  # [B, 128, KT, 2Q]

    if sum(nloads):
        encN16 = h_enc.astype(np.float16)  # [B, T, H]
        pieces = []
        for core in range(NCORES):
            for bl, nload in enumerate(nloads):
                if nload:
                    bglob = core * BL + bl
                    pieces.append(
                        tile_i(encN16[bglob : bglob + 1, : nload * 128, :], GC)[0]
                    )
        # [NCORES, sum(nloads)//GC, 128, GC*H]
        arrays["encN"] = np.ascontiguousarray(
            np.concatenate(pieces, 0).reshape(NCORES, -1, 128, GC * H)
        )
    return arrays


def _in_maps(arrays):
    maps = []
    for i in range(NCORES):
        m = {}
        for k, v in arrays.items():
            if k == "encN":
                m[k] = v[i]
            else:
                m[k] = v[i * BL : (i + 1) * BL]
        maps.append(m)
    return maps


def kernel(h_enc, h_dec, W_psi, b_psi, W_phi, b_phi):
    from concourse.bass_utils import run_bass_kernel_spmd

    arrays = _host_prep(h_enc, h_dec, W_psi, b_psi, W_phi, b_phi)
    nc = _build()
    res = run_bass_kernel_spmd(nc, _in_maps(arrays), core_ids=list(range(NCORES)))
    out = np.concatenate([res.results[i]["c"] for i in range(NCORES)], axis=0)
    return np.ascontiguousarray(out, dtype=np.float32)


# revision 27
# speedup vs baseline: 1.0172x; 1.0077x over previous
"""Trainium2 Bass kernel for batched bilinear (general) attention.

Reference computation (all fp32):
    psi = einsum("bth,ah->bta", h_enc, W_psi) + b_psi        # [B, T, A]
    phi = einsum("qbh,ah->qba", h_dec, W_phi) + b_phi        # [Q, B, A]
    e   = einsum("bta,qba->btq", psi, phi)                   # [B, T, Q]
    a   = softmax(e, axis=1)                                 # over T
    c   = einsum("bth,btq->bqh", h_enc, a)                   # [B, Q, H]

Algebraic refactor: e[b,t,q] = enc_t . M . dec_q + enc_t . u + (per-q const)
with M = W_psi^T @ W_phi [H,H], u = W_psi^T @ b_phi.  Per-q-column constants
are softmax-invariant and dropped.  The host folds weights into
Z[b] = M @ dec_b^T + u [H, Q] (tiny); the device computes e = enc @ Z,
softmax over T, and c = p^T @ enc.

Precision: e is computed as zh.enc16 + (zl.enc16)/SC with zh = f16(Z),
zl = f16((Z - zh)*SC) packed side by side in one [128-col] stationary
operand, so both channels come out of a single matmul pass (psum rows 0:64 =
main, 64:128 = correction).  Measured end-to-end rel-err 6e-3 vs the 2e-2
gate.

DMA-diet: enc is loaded from HBM ONCE per batch (f16, H-major "encT" form
for the e-phase).  The T-major copy needed by the c-phase is produced
on-chip by PE transposes hidden under the DMA stream for most tiles; only
`nloads[b]` of the 16 t-tiles per batch are loaded pre-transposed from HBM
(to balance PE vs DMA).  The last batch uses nloads=0 so its c-phase never
waits on DMA at the kernel tail.

Sharding: data-parallel over batch B=16 across 8 cores (2 per core).
"""

import functools
import os
import sys

import numpy as np

for _p in ("/opt/trn_rl_repo", "/root/.axon_site/_ro/trn_rl_repo"):
    if os.path.isdir(_p) and _p not in sys.path:
        sys.path.append(_p)

B, T, Q, H = 16, 2048, 64, 1024
NCORES = 8
BL = B // NCORES  # batches per core
KT = H // 128  # 8 contraction tiles for e
NT = T // 128  # 16 t-tiles
NC_CHUNK = T // 512  # 4 psum chunks along T for e
GE = int(os.environ.get("ATTN_GE", "1"))  # k-tiles per encT DMA transfer
GC = 2  # t-tiles per encN DMA (1 MB transfers, 4 KB contiguous/partition)
SC = 2048.0  # 2^11 scale for the zl correction channel

NLOADS = tuple(
    int(x) for x in os.environ.get("ATTN_NLOADS", "0,16").split(",")
)
DMA_SPREAD = int(os.environ.get("ATTN_DMA_SPREAD", "2"))
# SAFE=1: f32-PSUM transposes via regular matmul-by-identity (baseline-proven
# pattern); SAFE=0: f16-PSUM is_transpose path (faster, less PSUM)
SAFE = int(os.environ.get("ATTN_SAFE", "0"))


@functools.lru_cache(maxsize=4)
def _build(loop_n: int = 1, nloads: tuple = NLOADS, dma_spread: int = DMA_SPREAD, safe: int = SAFE):
    import contextlib

    import concourse.mybir as mybir
    import concourse.tile as tile
    from concourse import bacc
    from concourse.bass import ts
    from concourse.masks import make_identity

    f32 = mybir.dt.float32
    f16 = mybir.dt.float16

    nc = bacc.Bacc(
        "TRN2",
        target_bir_lowering=False,
        debug=False,
        enable_asserts=False,
        num_devices=NCORES,
    )

    encT_d = nc.dram_tensor(
        "encT", [BL, KT // GE, 128, GE * T], f16, kind="ExternalInput"
    )
    n_enc_dma = sum(nloads) // GC
    if n_enc_dma:
        encN_d = nc.dram_tensor(
            "encN", [n_enc_dma, 128, GC * H], f16, kind="ExternalInput"
        )
    z_d = nc.dram_tensor("z", [BL, 128, KT, 2 * Q], f16, kind="ExternalInput")
    c_d = nc.dram_tensor("c", [BL, Q, H], f32, kind="ExternalOutput")

    with tile.TileContext(nc) as tc:
        rings = [nc.sync, nc.scalar, nc.gpsimd, nc.vector][: max(2, dma_spread)]

        def dma(ring, out, in_):
            rings[ring % len(rings)].dma_start(out=out, in_=in_)

        with (
            tc.tile_pool(name="encT", bufs=int(os.environ.get("ATTN_ETBUFS", "16"))) as p_encT,
            tc.tile_pool(name="encN", bufs=2) as p_encN,
            tc.tile_pool(name="z", bufs=2) as p_z,
            tc.tile_pool(name="eT", bufs=2) as p_eT,
            tc.tile_pool(name="pT", bufs=2) as p_pT,
            tc.tile_pool(name="pN", bufs=2) as p_pN,
            tc.tile_pool(name="outs", bufs=2) as p_out,
            tc.tile_pool(name="stats", bufs=12) as p_stats,
            tc.tile_pool(name="singles", bufs=1) as p_singles,
            tc.tile_pool(name="ps", bufs=8, space="PSUM") as ps,
        ):
            ident128 = p_singles.tile([128, 128], f16)
            make_identity(nc, ident128)
            ident64 = p_singles.tile([64, 64], f32 if safe else f16)
            make_identity(nc, ident64)

            loop_ctx = (
                tc.For_i(0, loop_n, 1) if loop_n > 1 else contextlib.nullcontext()
            )
            with loop_ctx:
                z_ts, encN_sbs = [], []
                for b in range(BL):
                    z_t = p_z.tile([128, KT, 2 * Q], f16, tag="z")
                    dma(1, z_t[:], z_d.ap()[b])
                    z_ts.append(z_t)
                    encN_sb = p_encN.tile([128, NT, H], f16, tag="encN", name=f"encN_{b}")
                    encN_sbs.append(encN_sb)

                enc_dma_i = [0]

                def load_encN(b):
                    encN_sb = encN_sbs[b]
                    for i in range(nloads[b] // GC):
                        dma(
                            0,
                            encN_sb[:, GC * i : GC * (i + 1), :],
                            encN_d.ap()[enc_dma_i[0]],
                        )
                        enc_dma_i[0] += 1

                def phase_E(b):
                    """e^T[b] = Zpk[b]^T @ encT[b]; transpose ntr t-tiles of enc."""
                    z_t = z_ts[b]
                    encN_sb = encN_sbs[b]
                    nload = nloads[b]
                    ntr = NT - nload
                    e_pss = [
                        ps.tile([128, 512], f32, tag="ps", name=f"e_ps_{b}_{ci}")
                        for ci in range(NC_CHUNK)
                    ]
                    for kk in range(KT // GE):
                        encT_g = p_encT.tile([128, GE * T], f16, tag="encT")
                        dma(0, encT_g[:], encT_d.ap()[b, kk])
                        for g in range(GE):
                            k = kk * GE + g
                            for ci in range(NC_CHUNK):
                                nc.tensor.matmul(
                                    e_pss[ci][:],
                                    lhsT=z_t[:, k, :],
                                    rhs=encT_g[:, ts(g * NC_CHUNK + ci, 512)],
                                    start=(k == 0),
                                    stop=(k == KT - 1),
                                    skip_group_check=True,
                                )
                            # on-chip production of the T-major enc copy
                            gsz = 4 if safe else 8
                            for gi, h0 in enumerate(range(0, ntr, gsz)):
                                hn = min(gsz, ntr - h0)
                                tp = ps.tile(
                                    [128, hn, 128], f32 if safe else f16,
                                    tag="ps", name=f"tp_{b}_{k}_{h0}",
                                )
                                for j in range(hn):
                                    tt = nload + h0 + j
                                    src_ap = encT_g[
                                        :, g * T + tt * 128 : g * T + (tt + 1) * 128
                                    ]
                                    if safe:
                                        # out = encT_slice^T @ I = transposed
                                        # tile, in ordinary f32 PSUM
                                        nc.tensor.matmul(
                                            tp[:, j, :],
                                            lhsT=src_ap,
                                            rhs=ident128[:],
                                            start=True,
                                            stop=True,
                                            skip_group_check=True,
                                        )
                                    else:
                                        nc.tensor.matmul(
                                            tp[:, j, :],
                                            lhsT=src_ap,
                                            rhs=ident128[:],
                                            is_transpose=True,
                                            start=True,
                                            stop=True,
                                            skip_group_check=True,
                                        )
                                dst = encN_sb[
                                    :, nload + h0 : nload + h0 + hn, ts(k, 128)
                                ]
                                if (k + gi) % 2 == 0:
                                    nc.vector.tensor_copy(out=dst, in_=tp[:])
                                else:
                                    nc.scalar.copy(out=dst, in_=tp[:])
                    return e_pss

                def phase_S(b, e_pss):
                    """softmax stats over T; p in f16, correction folded in."""
                    eT = p_eT.tile([64, T], f32, tag="eT")
                    for ci in range(NC_CHUNK):
                        nc.scalar.activation(
                            out=eT[:, ts(ci, 512)],
                            in_=e_pss[ci][64:128, :],
                            func=mybir.ActivationFunctionType.Copy,
                            bias=0.0,
                            scale=1.0 / SC,
                        )
                        nc.vector.tensor_add(
                            eT[:, ts(ci, 512)], eT[:, ts(ci, 512)], e_pss[ci][0:64, :]
                        )
                    negm = p_stats.tile([64, 1], f32, tag="negm")
                    nc.vector.reduce_max(
                        out=negm[:], in_=eT[:], axis=mybir.AxisListType.X, negate=True
                    )
                    pT = p_pT.tile([64, T], f16 if not safe else f32, tag="pT")
                    s_sum = p_stats.tile([64, 1], f32, tag="s")
                    nc.scalar.activation(
                        out=pT[:],
                        in_=eT[:],
                        func=mybir.ActivationFunctionType.Exp,
                        bias=negm[:],
                        scale=1.0,
                        accum_out=s_sum[:],
                    )
                    r = p_stats.tile([64, 1], f32, tag="r")
                    nc.vector.reciprocal(out=r[:], in_=s_sum[:])
                    return pT, r

                def phase_C(b, pT, r):
                    """c[b] = (p^T @ encN) * r, transposed t-tiles first."""
                    nload = nloads[b]
                    encN_sb = encN_sbs[b]
                    pN = p_pN.tile([128, NT, Q], f16, tag="pN")
                    for tg in range(NT // 4):
                        trp = ps.tile(
                            [128, 4, Q], f32 if safe else f16,
                            tag="ps", name=f"trp_{b}_{tg}",
                        )
                        for j in range(4):
                            tt = tg * 4 + j
                            if safe:
                                nc.tensor.transpose(
                                    out=trp[:, j, :],
                                    in_=pT[:, ts(tt, 128)],
                                    identity=ident64[:],
                                )
                            else:
                                nc.tensor.matmul(
                                    trp[:, j, :],
                                    lhsT=pT[:, ts(tt, 128)],
                                    rhs=ident64[:],
                                    is_transpose=True,
                                    start=True,
                                    stop=True,
                                    skip_group_check=True,
                                )
                        nc.vector.tensor_copy(
                            out=pN[:, tg * 4 : (tg + 1) * 4, :], in_=trp[:]
                        )
                    # both H-halves accumulate in ONE psum bank: half 0 in
                    # partitions 0:64, half 1 in 64:128 via tile_position
                    c_ps = ps.tile([128, 512], f32, tag="ps", name=f"c_{b}")
                    order = list(range(nload, NT)) + list(range(nload))
                    for i, tt in enumerate(order):
                        nc.tensor.matmul(
                            c_ps[0:64, :],
                            lhsT=pN[:, tt, :],
                            rhs=encN_sb[:, tt, 0:512],
                            start=(i == 0),
                            stop=(i == NT - 1),
                            skip_group_check=True,
                        )
                        nc.tensor.matmul(
                            c_ps[64:128, :],
                            lhsT=pN[:, tt, :],
                            rhs=encN_sb[:, tt, 512:1024],
                            start=(i == 0),
                            stop=(i == NT - 1),
                            tile_position=(0, 64),
                            skip_group_check=True,
                        )
                    out_t = p_out.tile([64, H], f32, tag="out")
                    nc.vector.tensor_scalar_mul(out_t[:, 0:512], c_ps[0:64, :], r[:])
                    nc.vector.tensor_scalar_mul(
                        out_t[:, 512:1024], c_ps[64:128, :], r[:]
                    )
                    # Pool/gpsimd SWDGE ring: a waiting out-store here blocks
                    # nothing — under For_i an out on the sync ring would stall
                    # the NEXT iteration's encT loads behind it in SP's FIFO
                    nc.gpsimd.dma_start(out=c_d.ap()[b], in_=out_t[:])

                # PE warm-up: data-independent transposes ramp the tensor
                # engine to its max p-state while the first DMAs land
                n_warm = int(os.environ.get("ATTN_WARM", "24"))
                if n_warm:
                    warm_ps = ps.tile(
                        [128, 128], f32 if safe else f16, tag="ps", name="warm"
                    )
                    for _ in range(n_warm):
                        kw = {} if safe else {"is_transpose": True}
                        nc.tensor.matmul(
                            warm_ps[:],
                            lhsT=ident128[:],
                            rhs=ident128[:],
                            start=True,
                            stop=True,
                            skip_group_check=True,
                            **kw,
                        )

                # PE order: E0, E1, C0, C1 — softmax S(b) runs on ACT/DVE in
                # the shadow of the next phase's PE stream, so PE never stalls
                # on it.  DMA ring 0 order: encT b0, encT b1, encN b1.
                e_pss0 = phase_E(0)
                load_encN(0)
                pT0, r0 = phase_S(0, e_pss0)
                e_pss1 = phase_E(1)
                load_encN(1)
                phase_C(0, pT0, r0)
                pT1, r1 = phase_S(1, e_pss1)
                phase_C(1, pT1, r1)

    nc.compile()
    return nc


def _host_prep(h_enc, h_dec, W_psi, b_psi, W_phi, b_phi, nloads: tuple = NLOADS):
    h_enc = np.asarray(h_enc, dtype=np.float32)
    h_dec = np.asarray(h_dec, dtype=np.float32)
    W_psi = np.asarray(W_psi, dtype=np.float64)
    W_phi = np.asarray(W_phi, dtype=np.float64)
    b_phi = np.asarray(b_phi, dtype=np.float64)

    # M = W_psi^T @ W_phi [H, H];  u = W_psi^T @ b_phi [H]
    M = W_psi.T @ W_phi
    u = W_psi.T @ b_phi
    # Z[b, h, q] = sum_k M[h, k] * h_dec[q, b, k] + u[h]
    dec_r = h_dec.astype(np.float64).transpose(2, 1, 0).reshape(H, B * Q)
    Z = (M @ dec_r).reshape(H, B, Q).transpose(1, 0, 2) + u[None, :, None]
    Z = np.ascontiguousarray(Z, dtype=np.float32)  # [B, H, Q]

    def tile_i(x, g):  # [B, G*g*128, W] -> [B, G, 128, g*W] interleaved
        Bn, R, W = x.shape
        G = R // (g * 128)
        return np.ascontiguousarray(
            x.reshape(Bn, G, g, 128, W).transpose(0, 1, 3, 2, 4).reshape(
                Bn, G, 128, g * W
            )
        )

    encT = np.ascontiguousarray(h_enc.transpose(0, 2, 1))  # [B, H, T] fp32
    arrays = {"encT": tile_i(encT.astype(np.float16), GE)}

    zh = Z.astype(np.float16)
    zl = ((Z - zh.astype(np.float32)) * SC).astype(np.float16)
    zpk = np.concatenate([zh, zl], axis=2)  # [B, H, 2Q]
    arrays["z"] = np.ascontiguousarray(
        zpk.reshape(B, KT, 128, 2 * Q).transpose(0, 2, 1, 3)
    )  # [B, 128, KT, 2Q]

    if sum(nloads):
        encN16 = h_enc.astype(np.float16)  # [B, T, H]
        pieces = []
        for core in range(NCORES):
            for bl, nload in enumerate(nloads):
                if nload:
                    bglob = core * BL + bl
                    pieces.append(
                        tile_i(encN16[bglob : bglob + 1, : nload * 128, :], GC)[0]
                    )
        # [NCORES, sum(nloads)//GC, 128, GC*H]
        arrays["encN"] = np.ascontiguousarray(
            np.concatenate(pieces, 0).reshape(NCORES, -1, 128, GC * H)
        )
    return arrays


def _in_maps(arrays):
    maps = []
    for i in range(NCORES):
        m = {}
        for k, v in arrays.items():
            if k == "encN":
                m[k] = v[i]
            else:
                m[k] = v[i * BL : (i + 1) * BL]
        maps.append(m)
    return maps


def kernel(h_enc, h_dec, W_psi, b_psi, W_phi, b_phi):
    from concourse.bass_utils import run_bass_kernel_spmd

    arrays = _host_prep(h_enc, h_dec, W_psi, b_psi, W_phi, b_phi)
    nc = _build()
    res = run_bass_kernel_spmd(nc, _in_maps(arrays), core_ids=list(range(NCORES)))
    out = np.concatenate([res.results[i]["c"] for i in range(NCORES)], axis=0)
    return np.ascontiguousarray(out, dtype=np.float32)


# revision 28
# speedup vs baseline: 1.0205x; 1.0033x over previous
"""Trainium2 Bass kernel for batched bilinear (general) attention.

Reference computation (all fp32):
    psi = einsum("bth,ah->bta", h_enc, W_psi) + b_psi        # [B, T, A]
    phi = einsum("qbh,ah->qba", h_dec, W_phi) + b_phi        # [Q, B, A]
    e   = einsum("bta,qba->btq", psi, phi)                   # [B, T, Q]
    a   = softmax(e, axis=1)                                 # over T
    c   = einsum("bth,btq->bqh", h_enc, a)                   # [B, Q, H]

Algebraic refactor: e[b,t,q] = enc_t . M . dec_q + enc_t . u + (per-q const)
with M = W_psi^T @ W_phi [H,H], u = W_psi^T @ b_phi.  Per-q-column constants
are softmax-invariant and dropped.  The host folds weights into
Z[b] = M @ dec_b^T + u [H, Q] (tiny); the device computes e = enc @ Z,
softmax over T, and c = p^T @ enc.

Precision: e is computed as zh.enc16 + (zl.enc16)/SC with zh = f16(Z),
zl = f16((Z - zh)*SC) packed side by side in one [128-col] stationary
operand, so both channels come out of a single matmul pass (psum rows 0:64 =
main, 64:128 = correction).  Measured end-to-end rel-err 6e-3 vs the 2e-2
gate.

DMA-diet: enc is loaded from HBM ONCE per batch (f16, H-major "encT" form
for the e-phase).  The T-major copy needed by the c-phase is produced
on-chip by PE transposes hidden under the DMA stream for most tiles; only
`nloads[b]` of the 16 t-tiles per batch are loaded pre-transposed from HBM
(to balance PE vs DMA).  The last batch uses nloads=0 so its c-phase never
waits on DMA at the kernel tail.

Sharding: data-parallel over batch B=16 across 8 cores (2 per core).
"""

import functools
import os
import sys

import numpy as np

for _p in ("/opt/trn_rl_repo", "/root/.axon_site/_ro/trn_rl_repo"):
    if os.path.isdir(_p) and _p not in sys.path:
        sys.path.append(_p)

B, T, Q, H = 16, 2048, 64, 1024
NCORES = 8
BL = B // NCORES  # batches per core
KT = H // 128  # 8 contraction tiles for e
NT = T // 128  # 16 t-tiles
NC_CHUNK = T // 512  # 4 psum chunks along T for e
GE = int(os.environ.get("ATTN_GE", "1"))  # k-tiles per encT DMA transfer
GC = 2  # t-tiles per encN DMA (1 MB transfers, 4 KB contiguous/partition)
SC = 2048.0  # 2^11 scale for the zl correction channel

NLOADS = tuple(
    int(x) for x in os.environ.get("ATTN_NLOADS", "0,16").split(",")
)
DMA_SPREAD = int(os.environ.get("ATTN_DMA_SPREAD", "2"))
# SAFE=1: f32-PSUM transposes via regular matmul-by-identity (baseline-proven
# pattern); SAFE=0: f16-PSUM is_transpose path (faster, less PSUM)
SAFE = int(os.environ.get("ATTN_SAFE", "0"))


@functools.lru_cache(maxsize=4)
def _build(loop_n: int = 1, nloads: tuple = NLOADS, dma_spread: int = DMA_SPREAD, safe: int = SAFE):
    import contextlib

    import concourse.mybir as mybir
    import concourse.tile as tile
    from concourse import bacc
    from concourse.bass import ts
    from concourse.masks import make_identity

    f32 = mybir.dt.float32
    f16 = mybir.dt.float16

    nc = bacc.Bacc(
        "TRN2",
        target_bir_lowering=False,
        debug=False,
        enable_asserts=False,
        num_devices=NCORES,
    )

    encT_d = nc.dram_tensor(
        "encT", [BL, KT // GE, 128, GE * T], f16, kind="ExternalInput"
    )
    n_enc_dma = sum(nloads) // GC
    if n_enc_dma:
        encN_d = nc.dram_tensor(
            "encN", [n_enc_dma, 128, GC * H], f16, kind="ExternalInput"
        )
    z_d = nc.dram_tensor("z", [BL, 128, KT, 2 * Q], f16, kind="ExternalInput")
    c_d = nc.dram_tensor("c", [BL, Q, H], f32, kind="ExternalOutput")

    with tile.TileContext(nc) as tc:
        rings = [nc.sync, nc.scalar, nc.gpsimd, nc.vector][: max(2, dma_spread)]

        def dma(ring, out, in_):
            rings[ring % len(rings)].dma_start(out=out, in_=in_)

        with (
            tc.tile_pool(name="encT", bufs=int(os.environ.get("ATTN_ETBUFS", "16"))) as p_encT,
            tc.tile_pool(name="encN", bufs=2) as p_encN,
            tc.tile_pool(name="z", bufs=2) as p_z,
            tc.tile_pool(name="eT", bufs=2) as p_eT,
            tc.tile_pool(name="pT", bufs=2) as p_pT,
            tc.tile_pool(name="pN", bufs=2) as p_pN,
            tc.tile_pool(name="outs", bufs=2) as p_out,
            tc.tile_pool(name="stats", bufs=12) as p_stats,
            tc.tile_pool(name="singles", bufs=1) as p_singles,
            tc.tile_pool(name="ps", bufs=8, space="PSUM") as ps,
        ):
            ident128 = p_singles.tile([128, 128], f16)
            make_identity(nc, ident128)
            ident64 = p_singles.tile([64, 64], f32 if safe else f16)
            make_identity(nc, ident64)

            loop_ctx = (
                tc.For_i(0, loop_n, 1) if loop_n > 1 else contextlib.nullcontext()
            )
            with loop_ctx:
                z_ts, encN_sbs = [], []
                for b in range(BL):
                    z_t = p_z.tile([128, KT, 2 * Q], f16, tag="z")
                    dma(1, z_t[:], z_d.ap()[b])
                    z_ts.append(z_t)
                    encN_sb = p_encN.tile([128, NT, H], f16, tag="encN", name=f"encN_{b}")
                    encN_sbs.append(encN_sb)

                enc_dma_i = [0]

                def load_encN(b):
                    encN_sb = encN_sbs[b]
                    for i in range(nloads[b] // GC):
                        dma(
                            0,
                            encN_sb[:, GC * i : GC * (i + 1), :],
                            encN_d.ap()[enc_dma_i[0]],
                        )
                        enc_dma_i[0] += 1

                def phase_E(b):
                    """e^T[b] = Zpk[b]^T @ encT[b]; transpose ntr t-tiles of enc."""
                    z_t = z_ts[b]
                    encN_sb = encN_sbs[b]
                    nload = nloads[b]
                    ntr = NT - nload
                    e_pss = [
                        ps.tile([128, 512], f32, tag="ps", name=f"e_ps_{b}_{ci}")
                        for ci in range(NC_CHUNK)
                    ]
                    for kk in range(KT // GE):
                        encT_g = p_encT.tile([128, GE * T], f16, tag="encT")
                        dma(0, encT_g[:], encT_d.ap()[b, kk])
                        for g in range(GE):
                            k = kk * GE + g
                            for ci in range(NC_CHUNK):
                                nc.tensor.matmul(
                                    e_pss[ci][:],
                                    lhsT=z_t[:, k, :],
                                    rhs=encT_g[:, ts(g * NC_CHUNK + ci, 512)],
                                    start=(k == 0),
                                    stop=(k == KT - 1),
                                    skip_group_check=True,
                                )
                            # on-chip production of the T-major enc copy
                            gsz = 4 if safe else 8
                            for gi, h0 in enumerate(range(0, ntr, gsz)):
                                hn = min(gsz, ntr - h0)
                                tp = ps.tile(
                                    [128, hn, 128], f32 if safe else f16,
                                    tag="ps", name=f"tp_{b}_{k}_{h0}",
                                )
                                for j in range(hn):
                                    tt = nload + h0 + j
                                    src_ap = encT_g[
                                        :, g * T + tt * 128 : g * T + (tt + 1) * 128
                                    ]
                                    if safe:
                                        # out = encT_slice^T @ I = transposed
                                        # tile, in ordinary f32 PSUM
                                        nc.tensor.matmul(
                                            tp[:, j, :],
                                            lhsT=src_ap,
                                            rhs=ident128[:],
                                            start=True,
                                            stop=True,
                                            skip_group_check=True,
                                        )
                                    else:
                                        nc.tensor.matmul(
                                            tp[:, j, :],
                                            lhsT=src_ap,
                                            rhs=ident128[:],
                                            is_transpose=True,
                                            start=True,
                                            stop=True,
                                            skip_group_check=True,
                                        )
                                dst = encN_sb[
                                    :, nload + h0 : nload + h0 + hn, ts(k, 128)
                                ]
                                if (k + gi) % 2 == 0:
                                    nc.vector.tensor_copy(out=dst, in_=tp[:])
                                else:
                                    nc.scalar.copy(out=dst, in_=tp[:])
                    return e_pss

                def phase_S(b, e_pss):
                    """softmax stats over T; p in f16, correction folded in."""
                    eT = p_eT.tile([64, T], f32, tag="eT")
                    m4 = p_stats.tile([64, NC_CHUNK], f32, tag="m4")
                    for ci in range(NC_CHUNK):
                        nc.scalar.activation(
                            out=eT[:, ts(ci, 512)],
                            in_=e_pss[ci][64:128, :],
                            func=mybir.ActivationFunctionType.Copy,
                            bias=0.0,
                            scale=1.0 / SC,
                        )
                        nc.vector.tensor_add(
                            eT[:, ts(ci, 512)], eT[:, ts(ci, 512)], e_pss[ci][0:64, :]
                        )
                        # per-chunk max hides under the next chunk's ACT evac,
                        # shortening the serial chain to exp by ~1.5us
                        nc.vector.reduce_max(
                            out=m4[:, ci : ci + 1],
                            in_=eT[:, ts(ci, 512)],
                            axis=mybir.AxisListType.X,
                        )
                    negm = p_stats.tile([64, 1], f32, tag="negm")
                    nc.vector.reduce_max(
                        out=negm[:], in_=m4[:], axis=mybir.AxisListType.X, negate=True
                    )
                    pT = p_pT.tile([64, T], f16 if not safe else f32, tag="pT")
                    # exp in halves: pN transposes of the first half start while
                    # the second half is still on ACT
                    s2 = p_stats.tile([64, 2], f32, tag="s2")
                    for h in range(2):
                        nc.scalar.activation(
                            out=pT[:, h * (T // 2) : (h + 1) * (T // 2)],
                            in_=eT[:, h * (T // 2) : (h + 1) * (T // 2)],
                            func=mybir.ActivationFunctionType.Exp,
                            bias=negm[:],
                            scale=1.0,
                            accum_out=s2[:, h : h + 1],
                        )
                    s_sum = p_stats.tile([64, 1], f32, tag="s")
                    nc.vector.tensor_add(s_sum[:], s2[:, 0:1], s2[:, 1:2])
                    r = p_stats.tile([64, 1], f32, tag="r")
                    nc.vector.reciprocal(out=r[:], in_=s_sum[:])
                    return pT, r

                def phase_C(b, pT, r):
                    """c[b] = (p^T @ encN) * r, transposed t-tiles first."""
                    nload = nloads[b]
                    encN_sb = encN_sbs[b]
                    pN = p_pN.tile([128, NT, Q], f16, tag="pN")
                    for tg in range(NT // 4):
                        trp = ps.tile(
                            [128, 4, Q], f32 if safe else f16,
                            tag="ps", name=f"trp_{b}_{tg}",
                        )
                        for j in range(4):
                            tt = tg * 4 + j
                            if safe:
                                nc.tensor.transpose(
                                    out=trp[:, j, :],
                                    in_=pT[:, ts(tt, 128)],
                                    identity=ident64[:],
                                )
                            else:
                                nc.tensor.matmul(
                                    trp[:, j, :],
                                    lhsT=pT[:, ts(tt, 128)],
                                    rhs=ident64[:],
                                    is_transpose=True,
                                    start=True,
                                    stop=True,
                                    skip_group_check=True,
                                )
                        nc.vector.tensor_copy(
                            out=pN[:, tg * 4 : (tg + 1) * 4, :], in_=trp[:]
                        )
                    # both H-halves accumulate in ONE psum bank: half 0 in
                    # partitions 0:64, half 1 in 64:128 via tile_position
                    c_ps = ps.tile([128, 512], f32, tag="ps", name=f"c_{b}")
                    order = list(range(nload, NT)) + list(range(nload))
                    for i, tt in enumerate(order):
                        nc.tensor.matmul(
                            c_ps[0:64, :],
                            lhsT=pN[:, tt, :],
                            rhs=encN_sb[:, tt, 0:512],
                            start=(i == 0),
                            stop=(i == NT - 1),
                            skip_group_check=True,
                        )
                        nc.tensor.matmul(
                            c_ps[64:128, :],
                            lhsT=pN[:, tt, :],
                            rhs=encN_sb[:, tt, 512:1024],
                            start=(i == 0),
                            stop=(i == NT - 1),
                            tile_position=(0, 64),
                            skip_group_check=True,
                        )
                    out_t = p_out.tile([64, H], f32, tag="out")
                    nc.vector.tensor_scalar_mul(out_t[:, 0:512], c_ps[0:64, :], r[:])
                    nc.vector.tensor_scalar_mul(
                        out_t[:, 512:1024], c_ps[64:128, :], r[:]
                    )
                    # Pool/gpsimd SWDGE ring: a waiting out-store here blocks
                    # nothing — under For_i an out on the sync ring would stall
                    # the NEXT iteration's encT loads behind it in SP's FIFO
                    nc.gpsimd.dma_start(out=c_d.ap()[b], in_=out_t[:])

                # PE warm-up: data-independent transposes ramp the tensor
                # engine to its max p-state while the first DMAs land
                n_warm = int(os.environ.get("ATTN_WARM", "24"))
                if n_warm:
                    warm_ps = ps.tile(
                        [128, 128], f32 if safe else f16, tag="ps", name="warm"
                    )
                    for _ in range(n_warm):
                        kw = {} if safe else {"is_transpose": True}
                        nc.tensor.matmul(
                            warm_ps[:],
                            lhsT=ident128[:],
                            rhs=ident128[:],
                            start=True,
                            stop=True,
                            skip_group_check=True,
                            **kw,
                        )

                # PE order: E0, E1, C0, C1 — softmax S(b) runs on ACT/DVE in
                # the shadow of the next phase's PE stream, so PE never stalls
                # on it.  DMA ring 0 order: encT b0, encT b1, encN b1.
                e_pss0 = phase_E(0)
                load_encN(0)
                pT0, r0 = phase_S(0, e_pss0)
                e_pss1 = phase_E(1)
                load_encN(1)
                phase_C(0, pT0, r0)
                pT1, r1 = phase_S(1, e_pss1)
                phase_C(1, pT1, r1)

    nc.compile()
    return nc


def _host_prep(h_enc, h_dec, W_psi, b_psi, W_phi, b_phi, nloads: tuple = NLOADS):
    h_enc = np.asarray(h_enc, dtype=np.float32)
    h_dec = np.asarray(h_dec, dtype=np.float32)
    W_psi = np.asarray(W_psi, dtype=np.float64)
    W_phi = np.asarray(W_phi, dtype=np.float64)
    b_phi = np.asarray(b_phi, dtype=np.float64)

    # M = W_psi^T @ W_phi [H, H];  u = W_psi^T @ b_phi [H]
    M = W_psi.T @ W_phi
    u = W_psi.T @ b_phi
    # Z[b, h, q] = sum_k M[h, k] * h_dec[q, b, k] + u[h]
    dec_r = h_dec.astype(np.float64).transpose(2, 1, 0).reshape(H, B * Q)
    Z = (M @ dec_r).reshape(H, B, Q).transpose(1, 0, 2) + u[None, :, None]
    Z = np.ascontiguousarray(Z, dtype=np.float32)  # [B, H, Q]

    def tile_i(x, g):  # [B, G*g*128, W] -> [B, G, 128, g*W] interleaved
        Bn, R, W = x.shape
        G = R // (g * 128)
        return np.ascontiguousarray(
            x.reshape(Bn, G, g, 128, W).transpose(0, 1, 3, 2, 4).reshape(
                Bn, G, 128, g * W
            )
        )

    encT = np.ascontiguousarray(h_enc.transpose(0, 2, 1))  # [B, H, T] fp32
    arrays = {"encT": tile_i(encT.astype(np.float16), GE)}

    zh = Z.astype(np.float16)
    zl = ((Z - zh.astype(np.float32)) * SC).astype(np.float16)
    zpk = np.concatenate([zh, zl], axis=2)  # [B, H, 2Q]
    arrays["z"] = np.ascontiguousarray(
        zpk.reshape(B, KT, 128, 2 * Q).transpose(0, 2, 1, 3)
    )  # [B, 128, KT, 2Q]

    if sum(nloads):
        encN16 = h_enc.astype(np.float16)  # [B, T, H]
        pieces = []
        for core in range(NCORES):
            for bl, nload in enumerate(nloads):
                if nload:
                    bglob = core * BL + bl
                    pieces.append(
                        tile_i(encN16[bglob : bglob + 1, : nload * 128, :], GC)[0]
                    )
        # [NCORES, sum(nloads)//GC, 128, GC*H]
        arrays["encN"] = np.ascontiguousarray(
            np.concatenate(pieces, 0).reshape(NCORES, -1, 128, GC * H)
        )
    return arrays


def _in_maps(arrays):
    maps = []
    for i in range(NCORES):
        m = {}
        for k, v in arrays.items():
            if k == "encN":
                m[k] = v[i]
            else:
                m[k] = v[i * BL : (i + 1) * BL]
        maps.append(m)
    return maps


def kernel(h_enc, h_dec, W_psi, b_psi, W_phi, b_phi):
    from concourse.bass_utils import run_bass_kernel_spmd

    arrays = _host_prep(h_enc, h_dec, W_psi, b_psi, W_phi, b_phi)
    nc = _build()
    res = run_bass_kernel_spmd(nc, _in_maps(arrays), core_ids=list(range(NCORES)))
    out = np.concatenate([res.results[i]["c"] for i in range(NCORES)], axis=0)
    return np.ascontiguousarray(out, dtype=np.float32)
